# revision 8
# baseline (speedup 1.0000x reference)
"""LIF (leaky integrate-and-fire) scan kernel for Trainium2, 8 NeuronCores.

Reference semantics (fp32, T=8 innermost axis):
    mem = 0
    for t in range(T):
        mem = mem * 0.5 + x[..., t]
        s[..., t] = (mem >= 1.0)
        mem = mem * (1.0 - s[..., t])

Sharding: data-parallel over the leading dim (64 -> 8 per core).

Memory-roofline design: input must stream 32 MiB/core of fp32, but the
output is binary, so it leaves the device as uint8 (8 MiB/core instead of
32 MiB).  The host maps spikes back with (y == 1) -> f32, which is immune
to whether the device's f32->u8 conversion saturates or wraps.

Per-core layout is chunk-major/t-minor: x[p, (c*T + t)*CH + n] so every
strip the device touches is contiguous.  The per-timestep ops are split
across three engines so no engine exceeds the input-DMA time:
    A (DVE):    m = (r mult 0.5) add x_t          scalar_tensor_tensor
    B (Act):    y_t = Sign(m - 1) -> uint8        activation (sat to {0,1})
    C (GpSimd): r = (m is_lt 1) mult m            scalar_tensor_tensor
A is skipped at t=0 (mem0=0 -> m is just x_0) and C at t=T-1 (dead value).

The GpSimd (Pool) engine rejects scalar_tensor_tensor in codegen, so the
reset C is split: chunks with (c + t) even run it as one DVE stt; odd
ones run a two-op Pool pair (tensor_scalar is_lt -> gate, tensor_tensor
mult), balancing DVE ~95us / Pool ~97us / Act ~66us under the ~101us
input-DMA stream.

Input strips ride the otherwise-idle qSP hardware DGE queue (nc.sync);
output chunks ride qAct (nc.scalar).  Chunks are processed in groups of 3
with t-outer/chunk-inner issue order so the in-order engines always have
an independent chunk to work on while a chunk waits on the cross-engine
recurrence; the 27-deep input ring lets the DMA queue run a full group
ahead of compute.
"""

import numpy as np

import concourse.bass as bass
import concourse.tile as tile
from concourse import bacc, mybir
from concourse.bass_utils import run_bass_kernel_spmd

P = 128           # SBUF partitions
T = 8             # timesteps (innermost axis of the original input)
NPB = 8192        # neurons per partition per core: 8*128*32*32 / 128
CH = 1024         # neurons per chunk (per partition)
NCH = NPB // CH   # 8 chunks
GROUPS = [[0, 1, 2], [3, 4, 5], [6, 7]]

THRESH = 1.0
DECAY = 0.5
F32 = mybir.dt.float32
U8 = mybir.dt.uint8
N_CORES = 8

Alu = mybir.AluOpType
Act = mybir.ActivationFunctionType

# Spike op placement: "act" uses the Activation engine (Sign(m-1) saturated
# to u8); "dve" uses a 2x-mode tensor_scalar is_ge on the Vector engine.
B_ENGINE = "act"


def _build() -> bass.Bass:
    nc = bacc.Bacc("TRN2", target_bir_lowering=False, debug=False)
    x = nc.dram_tensor("x", [P, NCH * T * CH], F32, kind="ExternalInput").ap()
    y = nc.dram_tensor("y", [P, NCH * T * CH], U8, kind="ExternalOutput").ap()

    with tile.TileContext(nc) as tc:
        with (
            tc.tile_pool(name="consts", bufs=1) as cpool,
            tc.tile_pool(name="xs", bufs=27) as xpool,
            tc.tile_pool(name="ys", bufs=5) as ypool,
            tc.tile_pool(name="state", bufs=4) as spool,
            tc.tile_pool(name="gates", bufs=4) as gpool,
        ):
            neg_thresh = cpool.tile([P, 1], F32, tag="negth", name="neg_thresh")
            nc.gpsimd.memset(neg_thresh[:], -float(THRESH))

            def spike(out_ap, m_ap):
                if B_ENGINE == "act":
                    nc.scalar.activation(
                        out_ap, m_ap, Act.Sign, bias=neg_thresh[:]
                    )
                else:
                    nc.vector.tensor_scalar(
                        out_ap, m_ap, THRESH, None, Alu.is_ge, Alu.bypass
                    )

            def reset(c, t, src_ap, r_ap):
                # r = src * [src < 1].  DVE does it as one stt; Pool (which
                # rejects stt) as a tensor_scalar gate + tensor_tensor mult.
                if (c + t) % 2 == 0:
                    nc.vector.scalar_tensor_tensor(
                        r_ap, src_ap, THRESH, src_ap, Alu.is_lt, Alu.mult
                    )
                else:
                    g = gpool.tile([P, CH], F32, tag="g", name=f"g{c}_{t}")
                    nc.gpsimd.tensor_scalar(
                        g[:], src_ap, THRESH, None, Alu.is_lt, Alu.bypass
                    )
                    nc.gpsimd.tensor_tensor(r_ap, g[:], src_ap, Alu.mult)
            for chunks in GROUPS:
                # Input strips: t-outer / c-inner, all on the qSP HW queue.
                xs = {c: [None] * T for c in chunks}
                for t in range(T):
                    for c in chunks:
                        st = xpool.tile([P, CH], F32, tag="x", name=f"x{c}_{t}")
                        nc.sync.dma_start(
                            st[:], x[:, (c * T + t) * CH : (c * T + t + 1) * CH]
                        )
                        xs[c][t] = st

                yt, m, r = {}, {}, {}
                for c in chunks:
                    yt[c] = ypool.tile([P, T * CH], U8, tag="y", name=f"y{c}")
                    m[c] = spool.tile([P, CH], F32, tag="m", name=f"m{c}")
                    r[c] = spool.tile([P, CH], F32, tag="r", name=f"r{c}")

                # t = 0: mem0 = 0 so m == x_0; spike and reset read the strip.
                for c in chunks:
                    spike(yt[c][:, 0:CH], xs[c][0][:])
                for c in chunks:
                    reset(c, 0, xs[c][0][:], r[c][:])

                for t in range(1, T):
                    for c in chunks:
                        nc.vector.scalar_tensor_tensor(
                            m[c][:], r[c][:], DECAY, xs[c][t][:],
                            Alu.mult, Alu.add,
                        )
                    for c in chunks:
                        spike(yt[c][:, t * CH : (t + 1) * CH], m[c][:])
                    if t < T - 1:
                        for c in chunks:
                            reset(c, t, m[c][:], r[c][:])

                # Output: one contiguous u8 chunk per c on the qAct HW queue.
                for c in chunks:
                    nc.scalar.dma_start(
                        y[:, c * T * CH : (c + 1) * T * CH], yt[c][:]
                    )
    nc.compile()
    return nc


_NC_CACHE: bass.Bass | None = None


def _get_nc() -> bass.Bass:
    global _NC_CACHE
    if _NC_CACHE is None:
        _NC_CACHE = _build()
    return _NC_CACHE


def _run(X: np.ndarray, **spmd_kwargs):
    assert X.shape == (64, 128, 32, 32, 8), X.shape
    X = np.ascontiguousarray(X, dtype=np.float32)
    per_core = 64 // N_CORES
    # [core, p, nch, ch, t] -> chunk-major t-minor [core, p, nch, t, ch]
    Xt = np.ascontiguousarray(
        X.reshape(N_CORES, P, NCH, CH, T).transpose(0, 1, 2, 4, 3)
    )
    in_maps = [{"x": Xt[i].reshape(P, NCH * T * CH)} for i in range(N_CORES)]
    res = run_bass_kernel_spmd(
        _get_nc(), in_maps, core_ids=list(range(N_CORES)), **spmd_kwargs
    )
    out = np.empty_like(X)
    for i, r in enumerate(res.results):
        s = r["y"].reshape(P, NCH, T, CH).transpose(0, 1, 3, 2)
        out[i * per_core : (i + 1) * per_core] = (
            (s == 1).astype(np.float32).reshape(per_core, 128, 32, 32, 8)
        )
    return out, res


def kernel(X: np.ndarray) -> np.ndarray:
    out, _ = _run(X)
    return out


# revision 9
# speedup vs baseline: 2.0438x; 2.0438x over previous
"""LIF (leaky integrate-and-fire) scan kernel for Trainium2, 8 NeuronCores.

Reference semantics (fp32, T=8 innermost axis):
    mem = 0
    for t in range(T):
        mem = mem * 0.5 + x[..., t]
        s[..., t] = (mem >= 1.0)
        mem = mem * (1.0 - s[..., t])

Sharding: data-parallel over the leading dim (64 -> 8 per core).

Memory-roofline design: the input must stream 32 MiB/core of fp32, but the
output is binary, so it leaves the device as uint8 (8 MiB/core instead of
32 MiB).  The host maps spikes back with (y == 1) -> f32, immune to
whether the device's f32->u8 conversion saturates or wraps.

Per-core layout is chunk-major/t-minor: x[p, (c*T + t)*CH + n].  The
per-timestep ops are placed across FOUR engines by measured cost
(DVE stt 1.19us, DVE ts-2x 0.645us, Act 1.15us, Pool tt 3.18us, PE
identity-matmul A 3.2us per [128,1024] strip):

    A (m = 0.5 r + x):  DVE scalar_tensor_tensor for most steps; ~18
        steps/group-set run on the PE as two accumulating identity
        matmuls per 512-wide half (0.5I @ r then I @ x) into PSUM --
        exact, since each output is a single product.
    B (y_t = sign(m-1) -> u8): Act engine, reads SBUF or PSUM directly.
    C (r = m * [m < 1]): three flavours --
        stt:  one DVE scalar_tensor_tensor (SBUF m);
        pool: DVE tensor_scalar 2x gate + Pool tensor_tensor mult
              (Pool rejects stt and cannot read PSUM);
        PE-m: Act copies PSUM m to SBUF, then the pool flavour.

Engine-balance (measured): DVE ~95us, Act ~95us, Pool ~95us, PE ~57us,
all under/near the DMA floor.  Input strips split qSP (t=0..4) / qAct
(t=5..7) hardware DGE queues, issued in per-group prologues so ring-slot
waits never deadlock an engine sequencer; outputs ride qPool.
"""

import numpy as np

import concourse.bass as bass
import concourse.tile as tile
from concourse import bacc, mybir
from concourse.bass_utils import run_bass_kernel_spmd

P = 128           # SBUF partitions
T = 8             # timesteps (innermost axis of the original input)
NPB = 8192        # neurons per partition per core: 8*128*32*32 / 128
CH = 1024         # neurons per chunk (per partition)
NCH = NPB // CH   # 8 chunks
HMM = 512         # matmul moving-operand half width
GROUPS = [[0, 1, 2], [3, 4, 5], [6, 7]]

THRESH = 1.0
DECAY = 0.5
F32 = mybir.dt.float32
U8 = mybir.dt.uint8
N_CORES = 8

Alu = mybir.AluOpType
Act = mybir.ActivationFunctionType

QACT_T = 5        # strips with t >= QACT_T ride the qAct queue
N_DPOOL = 12      # D-step resets routed through the Pool engine


def _schedule():
    """Returns (a_pe, c_pool): sets of (c, t) steps."""
    a_pe = set()
    for chunks in GROUPS:
        n = len(chunks)
        for t in range(1, T):
            if n == 2 and t > 4:
                continue
            a_pe.add((chunks[(t - 1) % n], t))
    c_pool = set()
    d_cs = [
        (c, t)
        for chunks in GROUPS
        for t in range(T - 1)
        for c in chunks
        if (c, t) not in a_pe
    ]
    c_pool.update(d_cs[:: max(1, len(d_cs) // N_DPOOL)][:N_DPOOL])
    return a_pe, c_pool


def _build() -> bass.Bass:
    nc = bacc.Bacc("TRN2", target_bir_lowering=False, debug=False)
    x = nc.dram_tensor("x", [P, NCH * T * CH], F32, kind="ExternalInput").ap()
    w = nc.dram_tensor("w", [P, 256], F32, kind="ExternalInput").ap()
    y = nc.dram_tensor("y", [P, NCH * T * CH], U8, kind="ExternalOutput").ap()

    a_pe, c_pool = _schedule()

    with tile.TileContext(nc) as tc:
        with (
            tc.tile_pool(name="consts", bufs=1) as cpool,
            tc.tile_pool(name="xs", bufs=26) as xpool,
            tc.tile_pool(name="ys", bufs=5) as ypool,
            tc.tile_pool(name="ms", bufs=4) as mpool,
            tc.tile_pool(name="rs", bufs=4) as rpool,
            tc.tile_pool(name="mcopy", bufs=3) as mcpool,
            tc.tile_pool(name="gates", bufs=3) as gpool,
            tc.tile_pool(name="psum", bufs=4, space="PSUM") as ppool,
        ):
            neg_thresh = cpool.tile([P, 1], F32, tag="negth", name="neg_thresh")
            nc.gpsimd.memset(neg_thresh[:], -float(THRESH))
            wt = cpool.tile([P, 256], F32, tag="w", name="wt")
            nc.sync.dma_start(wt[:], w[:])
            w_id = wt[:, 0:128]
            w_half = wt[:, 128:256]

            def spike(c, t, m_ap):
                yslc = yt[c][:, t * CH : (t + 1) * CH]
                nc.scalar.activation(yslc, m_ap, Act.Sign, bias=neg_thresh[:])

            def reset(c, t, src_ap):
                # r[c] = src * [src < 1]
                if (c, t) in c_pool or (c, t) in a_pe:
                    g = gpool.tile([P, CH], F32, tag="g", name=f"g{c}_{t}")
                    nc.vector.tensor_scalar(
                        g[:], src_ap, THRESH, None, Alu.is_lt, Alu.bypass
                    )
                    nc.gpsimd.tensor_tensor(r[c][:], g[:], src_ap, Alu.mult)
                else:
                    nc.vector.scalar_tensor_tensor(
                        r[c][:], src_ap, THRESH, src_ap, Alu.is_lt, Alu.mult
                    )

            yt, r = {}, {}
            for chunks in GROUPS:
                # Prologue: this group's input strips.  t<QACT_T strips on
                # the qSP HW queue (SP never blocks); t>=QACT_T on qAct,
                # whose issue point (here, before this group's B ops) the
                # Act sequencer reaches while the previous group computes.
                xs = {c: [None] * T for c in chunks}
                for t in range(T):
                    for c in chunks:
                        st = xpool.tile([P, CH], F32, tag="x", name=f"x{c}_{t}")
                        eng = nc.sync if t < QACT_T else nc.scalar
                        eng.dma_start(
                            st[:], x[:, (c * T + t) * CH : (c * T + t + 1) * CH]
                        )
                        xs[c][t] = st

                for c in chunks:
                    yt[c] = ypool.tile([P, T * CH], U8, tag="y", name=f"y{c}")
                    r[c] = rpool.tile([P, CH], F32, tag="r", name=f"r{c}")

                # t = 0: mem0 = 0 so m == x_0 (SBUF strip).
                for c in chunks:
                    spike(c, 0, xs[c][0][:])
                for c in chunks:
                    reset(c, 0, xs[c][0][:])

                m = {}
                for t in range(1, T):
                    # A: m = 0.5*r + x_t
                    for c in chunks:
                        if (c, t) in a_pe:
                            pm = ppool.tile([P, CH], F32, tag="pm",
                                            name=f"pm{c}_{t}")
                            for h in range(2):
                                sl = slice(h * HMM, (h + 1) * HMM)
                                nc.tensor.matmul(
                                    pm[:, sl], w_half, r[c][:, sl],
                                    start=True, stop=False,
                                )
                                nc.tensor.matmul(
                                    pm[:, sl], w_id, xs[c][t][:, sl],
                                    start=False, stop=True,
                                )
                            m[c] = pm
                        else:
                            ms = mpool.tile([P, CH], F32, tag="m",
                                            name=f"m{c}_{t}")
                            nc.vector.scalar_tensor_tensor(
                                ms[:], r[c][:], DECAY, xs[c][t][:],
                                Alu.mult, Alu.add,
                            )
                            m[c] = ms
                    # B (+ PSUM->SBUF copy for PE steps' C)
                    for c in chunks:
                        spike(c, t, m[c][:])
                        if (c, t) in a_pe and t < T - 1:
                            mc = mcpool.tile([P, CH], F32, tag="mc",
                                             name=f"mc{c}_{t}")
                            nc.scalar.copy(mc[:], m[c][:])
                            m[c] = mc
                    # C
                    if t < T - 1:
                        for c in chunks:
                            reset(c, t, m[c][:])

                # Output: one contiguous u8 chunk per c on qPool.
                for c in chunks:
                    nc.gpsimd.dma_start(
                        y[:, c * T * CH : (c + 1) * T * CH], yt[c][:]
                    )
    nc.compile()
    return nc


_NC_CACHE: bass.Bass | None = None


def _get_nc() -> bass.Bass:
    global _NC_CACHE
    if _NC_CACHE is None:
        _NC_CACHE = _build()
    return _NC_CACHE


_W = np.concatenate(
    [np.eye(128, dtype=np.float32), 0.5 * np.eye(128, dtype=np.float32)], axis=1
)


def _run(X: np.ndarray, **spmd_kwargs):
    assert X.shape == (64, 128, 32, 32, 8), X.shape
    X = np.ascontiguousarray(X, dtype=np.float32)
    per_core = 64 // N_CORES
    # [core, p, nch, ch, t] -> chunk-major t-minor [core, p, nch, t, ch]
    Xt = np.ascontiguousarray(
        X.reshape(N_CORES, P, NCH, CH, T).transpose(0, 1, 2, 4, 3)
    )
    in_maps = [
        {"x": Xt[i].reshape(P, NCH * T * CH), "w": _W} for i in range(N_CORES)
    ]
    res = run_bass_kernel_spmd(
        _get_nc(), in_maps, core_ids=list(range(N_CORES)), **spmd_kwargs
    )
    out = np.empty_like(X)
    for i, rr in enumerate(res.results):
        s = rr["y"].reshape(P, NCH, T, CH).transpose(0, 1, 3, 2)
        out[i * per_core : (i + 1) * per_core] = (
            (s == 1).astype(np.float32).reshape(per_core, 128, 32, 32, 8)
        )
    return out, res


def kernel(X: np.ndarray) -> np.ndarray:
    out, _ = _run(X)
    return out


# revision 10
# speedup vs baseline: 3.2846x; 1.6072x over previous
"""LIF (leaky integrate-and-fire) scan kernel for Trainium2, 8 NeuronCores.

Reference semantics (fp32, T=8 innermost axis):
    mem = 0
    for t in range(T):
        mem = mem * 0.5 + x[..., t]
        s[..., t] = (mem >= 1.0)
        mem = mem * (1.0 - s[..., t])

Sharding: data-parallel over the leading dim (64 -> 8 per core).

Memory-roofline design: the input must stream 32 MiB/core of fp32, but the
output is binary, so it leaves the device as uint8 (8 MiB/core instead of
32 MiB).  The host maps spikes back with (y == 1) -> f32, immune to
whether the device's f32->u8 conversion saturates or wraps.

Per-core layout is chunk-major/t-minor: x[p, (c*T + t)*CH + n].  The
per-timestep ops are placed across FOUR engines by measured cost
(DVE stt 1.19us, DVE ts-2x 0.645us, Act 1.15us, Pool tt 3.18us, PE
identity-matmul A 3.2us per [128,1024] strip):

    A (m = 0.5 r + x):  DVE scalar_tensor_tensor for most steps; ~18
        steps/group-set run on the PE as two accumulating identity
        matmuls per 512-wide half (0.5I @ r then I @ x) into PSUM --
        exact, since each output is a single product.
    B (y_t = sign(m-1) -> u8): Act engine, reads SBUF or PSUM directly.
    C (r = m * [m < 1]): three flavours --
        stt:  one DVE scalar_tensor_tensor (SBUF m);
        pool: DVE tensor_scalar 2x gate + Pool tensor_tensor mult
              (Pool rejects stt and cannot read PSUM);
        PE-m: Act copies PSUM m to SBUF, then the pool flavour.

Engine-balance (measured): DVE ~95us, Act ~95us, Pool ~95us, PE ~57us,
all under/near the DMA floor.  Input strips split qSP (t=0..4) / qAct
(t=5..7) hardware DGE queues, issued in per-group prologues so ring-slot
waits never deadlock an engine sequencer; outputs ride qPool.
"""

import numpy as np

import concourse.bass as bass
import concourse.tile as tile
from concourse import bacc, mybir
from concourse.bass_utils import run_bass_kernel_spmd

P = 128           # SBUF partitions
T = 8             # timesteps (innermost axis of the original input)
NPB = 8192        # neurons per partition per core: 8*128*32*32 / 128
CH = 1024         # neurons per chunk (per partition)
NCH = NPB // CH   # 8 chunks
HMM = 512         # matmul moving-operand half width
GROUPS = [[0, 1, 2], [3, 4, 5], [6, 7]]

THRESH = 1.0
DECAY = 0.5
F32 = mybir.dt.float32
U8 = mybir.dt.uint8
N_CORES = 8

Alu = mybir.AluOpType
Act = mybir.ActivationFunctionType

QACT_T = 5        # strips with t >= QACT_T ride the qAct queue


def _schedule():
    """Returns (a_pe, c_pool): sets of (c, t) steps.

    Both empty: v3 measured that distributing the recurrence chain across
    PE/Pool/Act left every engine ~50% stalled on cross-engine latency
    (283us vs the 100us busy-time prediction).  The A->C->A chain now
    stays entirely on the DVE (in-order stream, chunk-interleaved, so the
    chain never waits), with only the terminal spike op B on Act.
    """
    return set(), set()


def _build() -> bass.Bass:
    nc = bacc.Bacc("TRN2", target_bir_lowering=False, debug=False)
    x = nc.dram_tensor("x", [P, NCH * T * CH], F32, kind="ExternalInput").ap()
    w = nc.dram_tensor("w", [P, 256], F32, kind="ExternalInput").ap()
    y = nc.dram_tensor("y", [P, NCH * T * CH], U8, kind="ExternalOutput").ap()

    a_pe, c_pool = _schedule()

    with tile.TileContext(nc) as tc:
        with (
            tc.tile_pool(name="consts", bufs=1) as cpool,
            tc.tile_pool(name="xs", bufs=26) as xpool,
            tc.tile_pool(name="ys", bufs=5) as ypool,
            tc.tile_pool(name="ms", bufs=4) as mpool,
            tc.tile_pool(name="rs", bufs=4) as rpool,
            tc.tile_pool(name="mcopy", bufs=3) as mcpool,
            tc.tile_pool(name="gates", bufs=3) as gpool,
            tc.tile_pool(name="psum", bufs=4, space="PSUM") as ppool,
        ):
            neg_thresh = cpool.tile([P, 1], F32, tag="negth", name="neg_thresh")
            nc.gpsimd.memset(neg_thresh[:], -float(THRESH))
            wt = cpool.tile([P, 256], F32, tag="w", name="wt")
            nc.sync.dma_start(wt[:], w[:])
            w_id = wt[:, 0:128]
            w_half = wt[:, 128:256]

            def spike(c, t, m_ap):
                yslc = yt[c][:, t * CH : (t + 1) * CH]
                nc.scalar.activation(yslc, m_ap, Act.Sign, bias=neg_thresh[:])

            def reset(c, t, src_ap):
                # r[c] = src * [src < 1]
                if (c, t) in c_pool or (c, t) in a_pe:
                    g = gpool.tile([P, CH], F32, tag="g", name=f"g{c}_{t}")
                    nc.vector.tensor_scalar(
                        g[:], src_ap, THRESH, None, Alu.is_lt, Alu.bypass
                    )
                    nc.gpsimd.tensor_tensor(r[c][:], g[:], src_ap, Alu.mult)
                else:
                    nc.vector.scalar_tensor_tensor(
                        r[c][:], src_ap, THRESH, src_ap, Alu.is_lt, Alu.mult
                    )

            yt, r = {}, {}
            for chunks in GROUPS:
                # Prologue: this group's input strips.  t<QACT_T strips on
                # the qSP HW queue (SP never blocks); t>=QACT_T on qAct,
                # whose issue point (here, before this group's B ops) the
                # Act sequencer reaches while the previous group computes.
                xs = {c: [None] * T for c in chunks}
                for t in range(T):
                    for c in chunks:
                        st = xpool.tile([P, CH], F32, tag="x", name=f"x{c}_{t}")
                        eng = nc.sync if t < QACT_T else nc.scalar
                        eng.dma_start(
                            st[:], x[:, (c * T + t) * CH : (c * T + t + 1) * CH]
                        )
                        xs[c][t] = st

                for c in chunks:
                    yt[c] = ypool.tile([P, T * CH], U8, tag="y", name=f"y{c}")
                    r[c] = rpool.tile([P, CH], F32, tag="r", name=f"r{c}")

                # t = 0: mem0 = 0 so m == x_0 (SBUF strip).
                for c in chunks:
                    spike(c, 0, xs[c][0][:])
                for c in chunks:
                    reset(c, 0, xs[c][0][:])

                m = {}
                for t in range(1, T):
                    # A: m = 0.5*r + x_t
                    for c in chunks:
                        if (c, t) in a_pe:
                            pm = ppool.tile([P, CH], F32, tag="pm",
                                            name=f"pm{c}_{t}")
                            for h in range(2):
                                sl = slice(h * HMM, (h + 1) * HMM)
                                nc.tensor.matmul(
                                    pm[:, sl], w_half, r[c][:, sl],
                                    start=True, stop=False,
                                )
                                nc.tensor.matmul(
                                    pm[:, sl], w_id, xs[c][t][:, sl],
                                    start=False, stop=True,
                                )
                            m[c] = pm
                        else:
                            ms = mpool.tile([P, CH], F32, tag="m",
                                            name=f"m{c}_{t}")
                            nc.vector.scalar_tensor_tensor(
                                ms[:], r[c][:], DECAY, xs[c][t][:],
                                Alu.mult, Alu.add,
                            )
                            m[c] = ms
                    # B (+ PSUM->SBUF copy for PE steps' C)
                    for c in chunks:
                        spike(c, t, m[c][:])
                        if (c, t) in a_pe and t < T - 1:
                            mc = mcpool.tile([P, CH], F32, tag="mc",
                                             name=f"mc{c}_{t}")
                            nc.scalar.copy(mc[:], m[c][:])
                            m[c] = mc
                    # C
                    if t < T - 1:
                        for c in chunks:
                            reset(c, t, m[c][:])

                # Output: one contiguous u8 chunk per c on qPool.
                for c in chunks:
                    nc.gpsimd.dma_start(
                        y[:, c * T * CH : (c + 1) * T * CH], yt[c][:]
                    )
    nc.compile()
    return nc


_NC_CACHE: bass.Bass | None = None


def _get_nc() -> bass.Bass:
    global _NC_CACHE
    if _NC_CACHE is None:
        _NC_CACHE = _build()
    return _NC_CACHE


_W = np.concatenate(
    [np.eye(128, dtype=np.float32), 0.5 * np.eye(128, dtype=np.float32)], axis=1
)


def _run(X: np.ndarray, **spmd_kwargs):
    assert X.shape == (64, 128, 32, 32, 8), X.shape
    X = np.ascontiguousarray(X, dtype=np.float32)
    per_core = 64 // N_CORES
    # [core, p, nch, ch, t] -> chunk-major t-minor [core, p, nch, t, ch]
    Xt = np.ascontiguousarray(
        X.reshape(N_CORES, P, NCH, CH, T).transpose(0, 1, 2, 4, 3)
    )
    in_maps = [
        {"x": Xt[i].reshape(P, NCH * T * CH), "w": _W} for i in range(N_CORES)
    ]
    res = run_bass_kernel_spmd(
        _get_nc(), in_maps, core_ids=list(range(N_CORES)), **spmd_kwargs
    )
    out = np.empty_like(X)
    for i, rr in enumerate(res.results):
        s = rr["y"].reshape(P, NCH, T, CH).transpose(0, 1, 3, 2)
        out[i * per_core : (i + 1) * per_core] = (
            (s == 1).astype(np.float32).reshape(per_core, 128, 32, 32, 8)
        )
    return out, res


def kernel(X: np.ndarray) -> np.ndarray:
    out, _ = _run(X)
    return out


# revision 11
# speedup vs baseline: 3.7759x; 1.1496x over previous
"""LIF (leaky integrate-and-fire) scan kernel for Trainium2, 8 NeuronCores.

Reference semantics (fp32, T=8 innermost axis):
    mem = 0
    for t in range(T):
        mem = mem * 0.5 + x[..., t]
        s[..., t] = (mem >= 1.0)
        mem = mem * (1.0 - s[..., t])

Sharding: data-parallel over the leading dim (64 -> 8 per core).

Memory-roofline design: the input must stream 32 MiB/core of fp32, but the
output is binary, so it leaves the device as uint8 (8 MiB/core instead of
32 MiB).  The host maps spikes back with (y == 1) -> f32, immune to
whether the device's f32->u8 conversion saturates or wraps.

Per-core layout is chunk-major/t-minor: x[p, (c*T + t)*CH + n].  The
per-timestep ops are placed across FOUR engines by measured cost
(DVE stt 1.19us, DVE ts-2x 0.645us, Act 1.15us, Pool tt 3.18us, PE
identity-matmul A 3.2us per [128,1024] strip):

    A (m = 0.5 r + x):  DVE scalar_tensor_tensor for most steps; ~18
        steps/group-set run on the PE as two accumulating identity
        matmuls per 512-wide half (0.5I @ r then I @ x) into PSUM --
        exact, since each output is a single product.
    B (y_t = sign(m-1) -> u8): Act engine, reads SBUF or PSUM directly.
    C (r = m * [m < 1]): three flavours --
        stt:  one DVE scalar_tensor_tensor (SBUF m);
        pool: DVE tensor_scalar 2x gate + Pool tensor_tensor mult
              (Pool rejects stt and cannot read PSUM);
        PE-m: Act copies PSUM m to SBUF, then the pool flavour.

Engine-balance (measured): DVE ~95us, Act ~95us, Pool ~95us, PE ~57us,
all under/near the DMA floor.  Input strips split qSP (t=0..4) / qAct
(t=5..7) hardware DGE queues, issued in per-group prologues so ring-slot
waits never deadlock an engine sequencer; outputs ride qPool.
"""

import numpy as np

import concourse.bass as bass
import concourse.tile as tile
from concourse import bacc, mybir
from concourse.bass_utils import run_bass_kernel_spmd

P = 128           # SBUF partitions
T = 8             # timesteps (innermost axis of the original input)
NPB = 8192        # neurons per partition per core: 8*128*32*32 / 128
CH = 1024         # neurons per chunk (per partition)
NCH = NPB // CH   # 8 chunks
HMM = 512         # matmul moving-operand half width
GROUPS = [[0, 1, 2], [3, 4, 5], [6, 7]]

THRESH = 1.0
DECAY = 0.5
F32 = mybir.dt.float32
U8 = mybir.dt.uint8
N_CORES = 8

Alu = mybir.AluOpType
Act = mybir.ActivationFunctionType

QACT_T = 5        # strips with t >= QACT_T ride the qAct queue


def _schedule():
    """Returns (a_pe, c_pool): sets of (c, t) steps.

    Both empty: v3 measured that distributing the recurrence chain across
    PE/Pool/Act left every engine ~50% stalled on cross-engine latency
    (283us vs the 100us busy-time prediction).  The A->C->A chain now
    stays entirely on the DVE (in-order stream, chunk-interleaved, so the
    chain never waits), with only the terminal spike op B on Act.
    """
    return set(), set()


def _build() -> bass.Bass:
    nc = bacc.Bacc("TRN2", target_bir_lowering=False, debug=False)
    x = nc.dram_tensor("x", [P, NCH * T * CH], F32, kind="ExternalInput").ap()
    w = nc.dram_tensor("w", [P, 256], F32, kind="ExternalInput").ap()
    y = nc.dram_tensor("y", [P, NCH * T * CH], U8, kind="ExternalOutput").ap()

    a_pe, c_pool = _schedule()

    with tile.TileContext(nc) as tc:
        with (
            tc.tile_pool(name="consts", bufs=1) as cpool,
            tc.tile_pool(name="xs", bufs=26) as xpool,
            tc.tile_pool(name="ys", bufs=5) as ypool,
            tc.tile_pool(name="ms", bufs=4) as mpool,
            tc.tile_pool(name="rs", bufs=4) as rpool,
            tc.tile_pool(name="mcopy", bufs=3) as mcpool,
            tc.tile_pool(name="gates", bufs=3) as gpool,
            tc.tile_pool(name="psum", bufs=4, space="PSUM") as ppool,
        ):
            neg_thresh = cpool.tile([P, 1], F32, tag="negth", name="neg_thresh")
            nc.gpsimd.memset(neg_thresh[:], -float(THRESH))
            wt = cpool.tile([P, 256], F32, tag="w", name="wt")
            nc.sync.dma_start(wt[:], w[:])
            w_id = wt[:, 0:128]
            w_half = wt[:, 128:256]

            def spike(c, t, m_ap):
                yslc = yt[c][:, t * CH : (t + 1) * CH]
                nc.scalar.activation(yslc, m_ap, Act.Sign, bias=neg_thresh[:])

            def reset(c, t, src_ap):
                # r[c] = src * [src < 1]
                if (c, t) in c_pool or (c, t) in a_pe:
                    g = gpool.tile([P, CH], F32, tag="g", name=f"g{c}_{t}")
                    nc.vector.tensor_scalar(
                        g[:], src_ap, THRESH, None, Alu.is_lt, Alu.bypass
                    )
                    nc.gpsimd.tensor_tensor(r[c][:], g[:], src_ap, Alu.mult)
                else:
                    nc.vector.scalar_tensor_tensor(
                        r[c][:], src_ap, THRESH, src_ap, Alu.is_lt, Alu.mult
                    )

            yt, r = {}, {}
            for chunks in GROUPS:
                # Prologue: this group's input strips.  t<QACT_T strips on
                # the qSP HW queue (SP never blocks); t>=QACT_T on qAct,
                # whose issue point (here, before this group's B ops) the
                # Act sequencer reaches while the previous group computes.
                xs = {c: [None] * T for c in chunks}
                for t in range(T):
                    for c in chunks:
                        st = xpool.tile([P, CH], F32, tag="x", name=f"x{c}_{t}")
                        # All input on qSP: the SP sequencer runs no compute,
                        # so strips always stream ahead of DVE consumption.
                        # (Strips routed via qAct stalled the last group 17us
                        # in v4: Act only issues them after the prior group's
                        # B ops.)
                        nc.sync.dma_start(
                            st[:], x[:, (c * T + t) * CH : (c * T + t + 1) * CH]
                        )
                        xs[c][t] = st

                for c in chunks:
                    yt[c] = ypool.tile([P, T * CH], U8, tag="y", name=f"y{c}")
                    r[c] = rpool.tile([P, CH], F32, tag="r", name=f"r{c}")

                # t = 0: mem0 = 0 so m == x_0 (SBUF strip).
                for c in chunks:
                    spike(c, 0, xs[c][0][:])
                for c in chunks:
                    reset(c, 0, xs[c][0][:])

                m = {}
                for t in range(1, T):
                    # A: m = 0.5*r + x_t
                    for c in chunks:
                        if (c, t) in a_pe:
                            pm = ppool.tile([P, CH], F32, tag="pm",
                                            name=f"pm{c}_{t}")
                            for h in range(2):
                                sl = slice(h * HMM, (h + 1) * HMM)
                                nc.tensor.matmul(
                                    pm[:, sl], w_half, r[c][:, sl],
                                    start=True, stop=False,
                                )
                                nc.tensor.matmul(
                                    pm[:, sl], w_id, xs[c][t][:, sl],
                                    start=False, stop=True,
                                )
                            m[c] = pm
                        else:
                            ms = mpool.tile([P, CH], F32, tag="m",
                                            name=f"m{c}_{t}")
                            nc.vector.scalar_tensor_tensor(
                                ms[:], r[c][:], DECAY, xs[c][t][:],
                                Alu.mult, Alu.add,
                            )
                            m[c] = ms
                    # B (+ PSUM->SBUF copy for PE steps' C)
                    for c in chunks:
                        spike(c, t, m[c][:])
                        if (c, t) in a_pe and t < T - 1:
                            mc = mcpool.tile([P, CH], F32, tag="mc",
                                             name=f"mc{c}_{t}")
                            nc.scalar.copy(mc[:], m[c][:])
                            m[c] = mc
                    # C
                    if t < T - 1:
                        for c in chunks:
                            reset(c, t, m[c][:])

                # Output: one contiguous u8 chunk per c on qPool.
                for c in chunks:
                    nc.gpsimd.dma_start(
                        y[:, c * T * CH : (c + 1) * T * CH], yt[c][:]
                    )
    nc.compile()
    return nc


_NC_CACHE: bass.Bass | None = None


def _get_nc() -> bass.Bass:
    global _NC_CACHE
    if _NC_CACHE is None:
        _NC_CACHE = _build()
    return _NC_CACHE


_W = np.concatenate(
    [np.eye(128, dtype=np.float32), 0.5 * np.eye(128, dtype=np.float32)], axis=1
)


def _run(X: np.ndarray, **spmd_kwargs):
    assert X.shape == (64, 128, 32, 32, 8), X.shape
    X = np.ascontiguousarray(X, dtype=np.float32)
    per_core = 64 // N_CORES
    # [core, p, nch, ch, t] -> chunk-major t-minor [core, p, nch, t, ch]
    Xt = np.ascontiguousarray(
        X.reshape(N_CORES, P, NCH, CH, T).transpose(0, 1, 2, 4, 3)
    )
    in_maps = [
        {"x": Xt[i].reshape(P, NCH * T * CH), "w": _W} for i in range(N_CORES)
    ]
    res = run_bass_kernel_spmd(
        _get_nc(), in_maps, core_ids=list(range(N_CORES)), **spmd_kwargs
    )
    out = np.empty_like(X)
    for i, rr in enumerate(res.results):
        s = rr["y"].reshape(P, NCH, T, CH).transpose(0, 1, 3, 2)
        out[i * per_core : (i + 1) * per_core] = (
            (s == 1).astype(np.float32).reshape(per_core, 128, 32, 32, 8)
        )
    return out, res


def kernel(X: np.ndarray) -> np.ndarray:
    out, _ = _run(X)
    return out
